# revision 36
# baseline (speedup 1.0000x reference)
"""Multi-head attention (RoPE, causal) Trainium2 Bass kernel — v2.

Sharding: 8 cores = DP(2 batches) x TP(4 head-quads of 4 heads each).
Each core computes, for its batch and head quad (2 pairs of 2 heads):
q/k projections + RoPE, v projection into a [vA | ones | vB] layout,
attention in "scoresT" orientation (scores[sk, sq]) where the ones
column block makes each ctx matmul also produce the softmax
denominators, and its partial slice of the output projection.  Host
sums the 4 TP partials per batch and adds bo.

v2 highlights vs v1:
  - ones-sums folded into the ctx matmul via [vA(64)|ones(64)|vB(64)]
    per-pair weight layout: one 128-col matmul per (head, sk-tile)
    instead of separate ctx + ones matmuls (-160 PE matmuls).
  - diagonal tiles only compute the allowed sq range (qoff = 128*i) and
    use a shared [128, 2*mw] duplicated triangle mask.
  - sc/e tiles are [128, 2, 512] so each (pair, sk-tile) needs one exp
    instruction even when the sq range is partial.
  - emission order overlaps pair-0 attention with pair-1 projections.
  - output partials written bf16 (halved output DMA).
"""

import sys

if "/opt/trn_rl_repo" not in sys.path:
    sys.path.insert(0, "/opt/trn_rl_repo")

import numpy as np
import ml_dtypes

import concourse.bass as bass
import concourse.bacc as bacc
import concourse.mybir as mybir
import concourse.tile as tile
from concourse.bass_utils import run_bass_kernel_spmd

BF16 = mybir.dt.bfloat16
F32 = mybir.dt.float32
NPBF16 = ml_dtypes.bfloat16

B, S, D, H, DK = 2, 2048, 1024, 16, 64
NCORES = 8
TP = 4            # head-quads per batch
HPC = H // TP     # heads per core = 4
OC = HPC * DK     # output dims per core for q/k/v projections = 256
NPAIR = HPC // 2  # head pairs per core = 2
NB = S // 512     # sq blocks of width 512
NT = S // 128     # sk tiles of width 128
ND = D // 128     # contraction d-tiles

last_exec_time_ns = None
_cache = {}


def _rope_tables():
    """COS/SSIN tables [128, S]: rows j in 0:32 = cos/-sin of freq j,
    rows 32:64 = cos/+sin, repeated for the 2nd head of the pair."""
    a = np.arange(0, DK, 2, dtype=np.float32)
    inv_freq = (10000.0 ** (-2.0 * a / DK)).astype(np.float32)  # [32]
    pos = np.arange(S, dtype=np.float32)
    ang = pos[:, None] * inv_freq[None, :]          # [S, 32]
    cos = np.cos(ang).T.astype(np.float32)          # [32, S]
    sin = np.sin(ang).T.astype(np.float32)
    cos128 = np.concatenate([cos, cos, cos, cos], axis=0)     # [128, S]
    # signs baked per-row for the shifted-output t2 formulation:
    # t2[e-rows] reads ssin[o-rows] -> needs -sin; t2[o-rows] reads
    # ssin[e-rows] -> needs +sin.
    sin128 = np.concatenate([sin, -sin, sin, -sin], axis=0)   # [128, S]
    return cos128, sin128


def _analyze_mask(mask):
    """Classify [sk_tile 128] x [sq_block 512] blocks of the attention mask.

    Returns (blocks, mask_tiles):
      blocks[b] = list of (t, qoff, mw, moff, mid) for sk tiles with any
      allowed element.  The unit computes sq columns [qoff, 512) of the
      block; if mid is not None, e columns [moff, moff+mw) relative to
      qoff (in both head planes) are multiplied by mask_tiles[mid]
      ([128 sk, mw], stored duplicated as [128, 2*mw]).
    """
    m = np.asarray(mask).reshape(S, S)  # [sq, sk], nonzero = allowed
    blocks = []
    tiles = []
    keys = {}
    for b in range(NB):
        cur = []
        for t in range(NT):
            sub = (m[512 * b:512 * b + 512, 128 * t:128 * t + 128] != 0)
            if not sub.any():
                continue
            col_any = sub.any(axis=1)
            col_all = sub.all(axis=1)
            q0 = int(np.nonzero(col_any)[0][0]) // 128 * 128
            part = col_any & ~col_all
            if part.any():
                pi = np.nonzero(part)[0]
                m0 = int(pi[0]) // 128 * 128
                m1 = -((-int(pi[-1]) - 1) // 128) * 128
                mw = m1 - m0
                tl = np.ascontiguousarray(sub[m0:m1, :].T).astype(NPBF16)
                k = tl.tobytes()
                if k not in keys:
                    keys[k] = len(tiles)
                    tiles.append(tl)
                cur.append((t, q0, mw, m0 - q0, keys[k]))
            else:
                cur.append((t, q0, 0, 0, None))
        blocks.append(cur)
    return blocks, tiles


def _build_nc(blocks, mask_tiles_meta, qk_bias=False, v_bias=False,
              loop_n=None):
    """mask_tiles_meta: list of widths (mw) of the mask tiles."""
    nc = bacc.Bacc(None)
    nm = max(len(mask_tiles_meta), 1)

    xq = nc.declare_dram_parameter("xqT", [D, S], BF16, isOutput=False)
    xk = nc.declare_dram_parameter("xkT", [D, S], BF16, isOutput=False)
    xv = nc.declare_dram_parameter("xvT", [D, S], BF16, isOutput=False)
    wq = nc.declare_dram_parameter("wqT", [D, OC], BF16, isOutput=False)
    wk = nc.declare_dram_parameter("wkT", [D, OC], BF16, isOutput=False)
    wv = nc.declare_dram_parameter("wvT", [D, OC], BF16, isOutput=False)
    wo = nc.declare_dram_parameter("woT", [OC, D], BF16, isOutput=False)
    cosd = nc.declare_dram_parameter("cos", [128, S], BF16, isOutput=False)
    ssind = nc.declare_dram_parameter("ssin", [128, S], BF16, isOutput=False)
    bqd = nc.declare_dram_parameter("bq", [128, NPAIR], F32, isOutput=False)
    bkd = nc.declare_dram_parameter("bk", [128, NPAIR], F32, isOutput=False)
    bvd = nc.declare_dram_parameter("bv", [128, OC], F32, isOutput=False)
    maskd = nc.declare_dram_parameter("masks", [nm, 128, 1024], BF16,
                                      isOutput=False)
    outp = nc.declare_dram_parameter("out", [S, D], BF16, isOutput=True)

    with tile.TileContext(nc) as tc:
        from contextlib import ExitStack
        with ExitStack() as ctx:
            ep = ctx.enter_context
            const = ep(tc.tile_pool(name="const", bufs=1))
            xt_p = ep(tc.tile_pool(name="xt", bufs=24))
            w_p = ep(tc.tile_pool(name="w", bufs=24))
            rope_p = ep(tc.tile_pool(name="rope", bufs=4))
            hat_p = ep(tc.tile_pool(name="hat", bufs=4))
            vsb_p = ep(tc.tile_pool(name="vsb", bufs=17))
            e_p = ep(tc.tile_pool(name="e", bufs=6))
            ctx_p = ep(tc.tile_pool(name="ctxsb", bufs=6))
            rec_p = ep(tc.tile_pool(name="rec", bufs=1))
            out_p = ep(tc.tile_pool(name="outsb", bufs=4))
            sc_ps = ep(tc.tile_pool(name="sc", bufs=2, space="PSUM"))
            ab_ps = ep(tc.tile_pool(name="ab", bufs=1, space="PSUM"))
            acc_ps = ep(tc.tile_pool(name="acc", bufs=2, space="PSUM"))
            if loop_n is not None:
                ep(tc.For_i(0, loop_n, 1))

            # warm the ACT exp table set early so the ~2.7us load overlaps
            # the projection phase instead of stalling the first real exp
            warm = const.tile([128, 32], F32, tag="warm")
            nc.any.memset(warm, 0.0)
            warm2 = const.tile([128, 32], BF16, tag="warm2")
            nc.scalar.activation(warm2, warm,
                                 mybir.ActivationFunctionType.Exp)

            def qk_proj(name, bias_sb, p):
                """Projection + RoPE, chunked per 512-col sb block so the
                DVE work pipelines with the PE accumulation chains."""
                xt = xts[name]
                raw = rope_p.tile([128, S], BF16, tag="raw")
                t1 = hat_p.tile([128, S], BF16, tag="hat")
                t2 = rope_p.tile([128, S], BF16, tag="t2")
                for sb in range(4):
                    ps = acc_ps.tile([128, 512], F32, tag="acc")
                    for dt in range(ND):
                        nc.tensor.matmul(
                            ps,
                            lhsT=wts[name][dt][:, 128 * p:128 * p + 128],
                            rhs=xt[dt][:, 512 * sb:512 * sb + 512],
                            start=(dt == 0), stop=(dt == ND - 1))
                    sl = slice(512 * sb, 512 * sb + 512)
                    if qk_bias:
                        tmp = rope_p.tile([128, 512], F32, tag="btmp")
                        nc.vector.tensor_copy(tmp, ps)
                        nc.vector.tensor_scalar_add(
                            raw[:, sl], tmp, bias_sb[:, p:p + 1])
                    else:
                        nc.any.tensor_copy(raw[:, sl], ps)
                    # RoPE: hat[e] = raw[e]*cos - raw[o]*sin
                    #       hat[o] = raw[o]*cos + raw[e]*sin
                    # t2 written with partition-SHIFTED outputs (inputs stay
                    # aligned; sign baked into the ssin table rows).
                    nc.vector.tensor_mul(t1[:, sl], raw[:, sl], cos_sb[:, sl])
                    nc.vector.tensor_mul(t2[0:32, sl], raw[32:64, sl],
                                         ssin_sb[32:64, sl])
                    nc.vector.tensor_mul(t2[32:64, sl], raw[0:32, sl],
                                         ssin_sb[0:32, sl])
                    nc.vector.tensor_mul(t2[64:96, sl], raw[96:128, sl],
                                         ssin_sb[96:128, sl])
                    nc.vector.tensor_mul(t2[96:128, sl], raw[64:96, sl],
                                         ssin_sb[64:96, sl])
                    nc.vector.tensor_add(t1[:, sl], t1[:, sl], t2[:, sl])
                hats[(name, p)] = t1

            hats = {}
            wts = {}
            xts = {}

            def load_xw(name, xd, wd, queues=(nc.sync, nc.gpsimd)):
                xts[name] = []
                wts[name] = []
                for dt in range(ND):
                    w_t = w_p.tile([128, OC], BF16, tag="w")
                    queues[(dt + 1) % len(queues)].dma_start(
                        out=w_t, in_=wd[128 * dt:128 * dt + 128, :])
                    wts[name].append(w_t)
                for dt in range(ND):
                    x_t = xt_p.tile([128, S], BF16, tag="xt")
                    queues[dt % len(queues)].dma_start(
                        out=x_t, in_=xd[128 * dt:128 * dt + 128, :])
                    xts[name].append(x_t)

            # v loads fan out over 4 queues: v gates ctxones, and ACT/DVE
            # are idle this early so their queues are free
            load_xw("v", xv, wv,
                    queues=(nc.sync, nc.gpsimd, nc.scalar))
            # cos/ssin early (needed by first RoPE); wo/masks loaded later
            cos_sb = const.tile([128, S], BF16)
            ssin_sb = const.tile([128, S], BF16)
            nc.sync.dma_start(out=cos_sb, in_=cosd[:, :])
            nc.sync.dma_start(out=ssin_sb, in_=ssind[:, :])
            if qk_bias:
                bq_sb = const.tile([128, NPAIR], F32)
                bk_sb = const.tile([128, NPAIR], F32)
                nc.gpsimd.dma_start(out=bq_sb, in_=bqd[:, :])
                nc.gpsimd.dma_start(out=bk_sb, in_=bkd[:, :])
            else:
                bq_sb = bk_sb = None
            if v_bias:
                bv_sb = const.tile([128, OC], F32)
                nc.gpsimd.dma_start(out=bv_sb, in_=bvd[:, :])
            load_xw("q", xq, wq)
            load_xw("k", xk, wk)
            wo_sb = []
            for p in range(NPAIR):
                w_t = const.tile([128, D], BF16, tag=f"wo{p}")
                nc.gpsimd.dma_start(out=w_t, in_=wo[128 * p:128 * p + 128, :])
                wo_sb.append(w_t)
            mask_sb = []
            for i, mw in enumerate(mask_tiles_meta):
                m_t = const.tile([128, 1024], BF16, tag=f"mask{i}")
                nc.gpsimd.dma_start(out=m_t, in_=maskd[i])
                mask_sb.append(m_t)

            # ---- v projection first (gates ctxones) ----
            vsb = []  # vsb[t] = [128, 384] tile [A0|ones|B0|A1|ones|B1]
            for st in range(NT):
                ps = acc_ps.tile([128, 512], F32, tag="acc")
                for dt in range(ND):
                    nc.tensor.matmul(
                        ps[:, 0:OC],
                        lhsT=xts["v"][dt][:, 128 * st:128 * st + 128],
                        rhs=wts["v"][dt][:, :],
                        start=(dt == 0), stop=(dt == ND - 1))
                v_t = vsb_p.tile([128, 384], BF16, tag="vsb")
                if v_bias:
                    nc.vector.tensor_add(v_t[:, 0:64], ps[:, 0:64],
                                         bv_sb[:, 0:64])
                    nc.vector.tensor_add(v_t[:, 128:256], ps[:, 64:192],
                                         bv_sb[:, 64:192])
                    nc.vector.tensor_add(v_t[:, 320:384], ps[:, 192:256],
                                         bv_sb[:, 192:256])
                else:
                    nc.vector.tensor_copy(v_t[:, 0:64], ps[:, 0:64])
                    nc.vector.tensor_copy(v_t[:, 128:256], ps[:, 64:192])
                    nc.vector.tensor_copy(v_t[:, 320:384], ps[:, 192:256])
                nc.any.memset(v_t[:, 64:128], 1.0)
                nc.any.memset(v_t[:, 256:320], 1.0)
                vsb.append(v_t)

            # pair-0 q/k so pair-0 attention can start ASAP
            qk_proj("q", bq_sb, 0)
            qk_proj("k", bk_sb, 0)

            # ---- attention ----
            def attention(p, b):
                """Emit attention for pair p, block b; returns csb tile."""
                qh = hats[("q", p)]
                kh = hats[("k", p)]
                act = blocks[b]
                # ab: [128, 2, 512] f32; plane 0 = [dimsA|sumsA],
                # plane 1 = [sumsB|dimsB] (partition halves)
                ab = ab_ps.tile([128, 2, 512], F32, tag="ab")
                n = len(act)
                for gi, (t, qoff, mw, moff, mid) in enumerate(act):
                    qlen = 512 - qoff
                    sc = sc_ps.tile([128, 2, 512], F32, tag="sc")
                    e = e_p.tile([128, 2, 512], BF16, tag="e")
                    nc.tensor.matmul(
                        sc[:, 0:1, 0:qlen],
                        lhsT=kh[0:64, 128 * t:128 * t + 128],
                        rhs=qh[0:64, 512 * b + qoff:512 * b + 512],
                        start=True, stop=True, tile_position=(0, 0))
                    nc.tensor.matmul(
                        sc[:, 1:2, 0:qlen],
                        lhsT=kh[64:128, 128 * t:128 * t + 128],
                        rhs=qh[64:128, 512 * b + qoff:512 * b + 512],
                        start=True, stop=True, tile_position=(64, 0))
                    nc.scalar.activation(
                        e[:, :, 0:qlen], sc[:, :, 0:qlen],
                        mybir.ActivationFunctionType.Exp)
                    if mid is not None:
                        # gpsimd: DVE is loaded in the attention phase while
                        # Pool is idle after the input DMAs
                        nc.gpsimd.tensor_mul(
                            e[:, :, moff:moff + mw], e[:, :, moff:moff + mw],
                            mask_sb[mid][:, 0:2 * mw])
                    first = (gi == 0)
                    last = (gi == n - 1)
                    nc.tensor.matmul(
                        ab[:, 0:1, qoff:512],
                        lhsT=vsb[t][:, 192 * p:192 * p + 128],
                        rhs=e[:, 0:1, 0:qlen], start=first, stop=last,
                        skip_group_check=True)
                    nc.tensor.matmul(
                        ab[:, 1:2, qoff:512],
                        lhsT=vsb[t][:, 192 * p + 64:192 * p + 192],
                        rhs=e[:, 1:2, 0:qlen], start=first, stop=last,
                        skip_group_check=True)
                rec = rec_p.tile([128, 512], F32, tag="rec")
                nc.vector.reciprocal(rec[0:64, :], ab[64:128, 0:1, :])
                nc.vector.reciprocal(rec[64:128, :], ab[0:64, 1:2, :])
                csb = ctx_p.tile([128, 512], BF16, tag="ctxsb")
                nc.vector.tensor_mul(csb[0:64, :], ab[0:64, 0:1, :],
                                     rec[0:64, :])
                nc.vector.tensor_mul(csb[64:128, :], ab[64:128, 1:2, :],
                                     rec[64:128, :])
                return csb

            def out_proj(b, ctx0, ctx1):
                for j in range(4):
                    for oh in range(2):
                        ps = acc_ps.tile([128, 512], F32, tag="acc")
                        for p, csb in ((0, ctx0), (1, ctx1)):
                            nc.tensor.matmul(
                                ps,
                                lhsT=csb[:, 128 * j:128 * j + 128],
                                rhs=wo_sb[p][:, 512 * oh:512 * oh + 512],
                                start=(p == 0), stop=(p == NPAIR - 1))
                        o_t = out_p.tile([128, 512], BF16, tag="outsb")
                        if (j + oh) % 2 == 0:
                            nc.vector.tensor_copy(o_t, ps)
                        else:
                            nc.scalar.copy(o_t, ps)
                        oq = nc.sync if (j + oh) % 2 == 0 else nc.gpsimd
                        oq.dma_start(
                            out=outp[512 * b + 128 * j:512 * b + 128 * j + 128,
                                     512 * oh:512 * oh + 512],
                            in_=o_t)

            csb0 = [attention(0, b) for b in range(NB)]
            qk_proj("q", bq_sb, 1)
            qk_proj("k", bk_sb, 1)
            # out_proj(b) emitted after attention(1, b+1) so its matmuls
            # have lower priority and act as filler for exp-wait bubbles
            csb1 = []
            for b in range(NB):
                csb1.append(attention(1, b))
                if b >= 1:
                    out_proj(b - 1, csb0[b - 1], csb1[b - 1])
            out_proj(NB - 1, csb0[NB - 1], csb1[NB - 1])
    nc.finalize()
    return nc


def _prep_core_inputs(inputs, blocks, mask_tiles):
    """Build the 8 per-core input maps (host-side sharding)."""
    q = np.asarray(inputs["q"], np.float32)
    k = np.asarray(inputs["k"], np.float32)
    v = np.asarray(inputs["v"], np.float32)
    Wq = np.asarray(inputs["Wq"], np.float32)
    Wk = np.asarray(inputs["Wk"], np.float32)
    Wv = np.asarray(inputs["Wv"], np.float32)
    Wo = np.asarray(inputs["Wo"], np.float32)
    bq = np.asarray(inputs["bq"], np.float32)
    bk = np.asarray(inputs["bk"], np.float32)
    bv = np.asarray(inputs["bv"], np.float32)

    cos128, ssin128 = _rope_tables()
    cos_b = cos128.astype(NPBF16)
    ssin_b = ssin128.astype(NPBF16)
    nm = max(len(mask_tiles), 1)
    masks_t = np.zeros((nm, 128, 1024), NPBF16)
    for i, t in enumerate(mask_tiles):
        w = t.shape[1]
        masks_t[i, :, 0:w] = t
        masks_t[i, :, w:2 * w] = t  # duplicated for the 2-head plane op

    # de-interleave permutation within each head: evens then odds
    perm64 = np.concatenate([np.arange(0, DK, 2), np.arange(1, DK, 2)])

    xT = {}
    for bb in range(B):
        xT[("q", bb)] = np.ascontiguousarray(q[bb].T).astype(NPBF16)
        xT[("k", bb)] = np.ascontiguousarray(k[bb].T).astype(NPBF16)
        xT[("v", bb)] = np.ascontiguousarray(v[bb].T).astype(NPBF16)

    scale = np.float32(1.0 / np.sqrt(DK))
    in_maps = []
    for c in range(NCORES):
        bb, hq = divmod(c, TP)
        rows = []
        for h in range(HPC):
            base = OC * hq + DK * h
            rows.extend((base + perm64).tolist())
        rows = np.array(rows)
        cols = np.arange(OC * hq, OC * hq + OC)

        wqT = np.ascontiguousarray(Wq[rows, :].T).astype(NPBF16)
        wkT = np.ascontiguousarray((Wk[rows, :] * scale).T).astype(NPBF16)
        wvT = np.ascontiguousarray(Wv[cols, :].T).astype(NPBF16)
        woT = np.ascontiguousarray(Wo[:, cols].T).astype(NPBF16)
        bq_t = np.ascontiguousarray(bq[rows].reshape(NPAIR, 128).T).astype(np.float32)
        bk_t = np.ascontiguousarray((bk[rows] * scale).reshape(NPAIR, 128).T).astype(np.float32)
        bv_t = np.broadcast_to(bv[cols], (128, OC)).astype(np.float32)

        in_maps.append({
            "xqT": xT[("q", bb)], "xkT": xT[("k", bb)], "xvT": xT[("v", bb)],
            "wqT": wqT, "wkT": wkT, "wvT": wvT, "woT": woT,
            "cos": cos_b, "ssin": ssin_b,
            "bq": bq_t, "bk": bk_t, "bv": bv_t,
            "masks": masks_t,
        })
    return in_maps


def kernel(**inputs):
    global last_exec_time_ns
    import os

    mask = np.asarray(inputs["mask"])
    blocks, mask_tiles = _analyze_mask(mask)
    qk_bias = bool(np.any(np.asarray(inputs["bq"])) or np.any(np.asarray(inputs["bk"])))
    v_bias = bool(np.any(np.asarray(inputs["bv"])))
    meta = tuple(t.shape[1] for t in mask_tiles)
    key = (tuple(tuple(bl) for bl in blocks), meta, qk_bias, v_bias)
    if key not in _cache:
        _cache[key] = _build_nc(blocks, list(meta), qk_bias, v_bias)
    nc = _cache[key]

    in_maps = _prep_core_inputs(inputs, blocks, mask_tiles)
    trace = bool(os.environ.get("KERNEL_TRACE"))
    import time
    last_err = None
    for attempt in range(5):
        try:
            res = run_bass_kernel_spmd(nc, in_maps, list(range(NCORES)),
                                       trace=trace)
            break
        except Exception as e:  # transient NRT device-unrecoverable wedges
            last_err = e
            time.sleep(10.0 + 10.0 * attempt)
    else:
        raise last_err
    last_exec_time_ns = res.exec_time_ns

    bo = np.asarray(inputs["bo"], np.float32)
    out = np.zeros((B, S, D), np.float32)
    for c in range(NCORES):
        bb = c // TP
        out[bb] += np.asarray(res.results[c]["out"], np.float32)
    out += bo[None, None, :]
    return out
